# revision 1
# baseline (speedup 1.0000x reference)
"""Trainium2 Bass kernel for GPUTimeMask: zero out per-batch time windows.

Semantics (matches reference):
    out = x.copy();  for m, b:  out[b, :, s[m,b] : s[m,b]+clip(w[m,b],1,150)] = 0

Strategy:
  - Shard x along the CHANNEL axis: 16 channels -> 2 per core across 8 cores.
    Every core then holds ALL 64 batch rows, so the (runtime-valued) mask
    windows live at identical local coordinates on every core -> one SPMD
    program with window offsets specialized in at build time.
  - Per core the work is a pure HBM->SBUF->HBM streaming copy of a
    [128, 60000] f32 plane (rows = batch*2 + local_channel) with ~130 tiny
    SBUF memsets (<= 2 partitions x 150 cols each) applied between load and
    store. The memsets hide entirely under the DMA stream, so the kernel
    runs at the memcpy roofline. No cross-core communication.
  - Programs are cached keyed on (starts, widths) bytes, so repeated calls
    with identical metadata skip rebuild/recompile.
"""

import sys

import numpy as np

for _p in ("/opt/trn_rl_repo",):
    if _p not in sys.path:
        sys.path.insert(0, _p)

import concourse.bass as bass
import concourse.mybir as mybir
from concourse.bass_utils import run_bass_kernel_spmd
from concourse.tile import TileContext
from concourse.tile_rust import add_dep_helper

B, C, T = 64, 16, 60000
MAX_MASK_WIDTH = 150
N_CORES = 8
C_LOCAL = C // N_CORES          # 2 channels per core
P = B * C_LOCAL                 # 128 partitions: row = b * C_LOCAL + c_local
# Middle tiles are [128, 7500] f32: 30 KB contiguous per partition per DMA
# packet.  Smaller packets hit a per-queue descriptor-dispatch ceiling
# (~310 GB/s at 10 KB); 30 KB packets sustain the full ~435 GB/s HBM duplex
# rate.  Small tiles at the START let the first store join the DMA-engine
# mix within a few us (reads-only runs at ~360 GB/s, mixed at ~435); small
# tiles at the END shorten the store-only drain after the last load.
_cols = [3750] + [7500] * 7 + [1875, 1875]
assert sum(_cols) == T
TILE_W = max(_cols)
TILE_RANGES = []
_off = 0
for _w in _cols:
    TILE_RANGES.append((_off, _off + _w))
    _off += _w
N_BUFS = 6

_program_cache: dict[bytes, bass.Bass] = {}


def _build_program(windows: list[tuple[int, int, int]]) -> bass.Bass:
    """windows: (b, lo, hi) global column ranges to zero; identical per core.

    Structure (DMA waits stall the ISSUING sequencer on this hardware, so
    waits must stay off the load path):
      - Loads stream on the sync HWDGE queue; the SP sequencer's only waits
        are buffer-reuse WARs that the queue's own progress pre-satisfies.
      - Mask windows are zeroed in SBUF by vector-engine tensor_scalar
        multiplies with a per-partition 0/1 selector (compute engines need
        32-aligned partition bases, so each op covers a 32-partition slab).
      - Stores issue from the Activation HWDGE queue; that sequencer absorbs
        the per-tile DVE waits without blocking load issue, and stores join
        the DMA-engine mix early (HBM runs ~435 GB/s only with reads and
        writes mixed; ~360 GB/s read-only).
    """
    nc = bass.Bass()
    x = nc.declare_dram_parameter("x", [P, T], mybir.dt.float32, isOutput=False)
    y = nc.declare_dram_parameter("y", [P, T], mybir.dt.float32, isOutput=True)
    with TileContext(nc) as tc:
        with (
            tc.tile_pool(name="const", bufs=1) as cpool,
            tc.tile_pool(name="io", bufs=N_BUFS) as pool,
        ):
            # sel[p, b] = 0.0 if p//C_LOCAL == b else 1.0, built on gpsimd
            # (the only engine with affine_select); one DVE touch then keeps
            # the cross-engine wait off the per-window fixup ops.
            sel_t = cpool.tile([P, B], mybir.dt.float32)
            tmp_t = cpool.tile([P, B], mybir.dt.float32)
            nc.gpsimd.memset(sel_t[:], 1.0)
            nc.gpsimd.memset(tmp_t[:], 1.0)
            nc.gpsimd.affine_select(
                sel_t[:], sel_t[:], [[-C_LOCAL, B]],
                mybir.AluOpType.is_ge, 0.0,
                base=-C_LOCAL, channel_multiplier=1,
            )
            # p < C_LOCAL*b  <=>  C_LOCAL*b - p - 1 >= 0  (is_lt unimplemented)
            nc.gpsimd.affine_select(
                tmp_t[:], tmp_t[:], [[C_LOCAL, B]],
                mybir.AluOpType.is_ge, 0.0,
                base=-1, channel_multiplier=-1,
            )
            nc.gpsimd.tensor_tensor(
                sel_t[:], sel_t[:], tmp_t[:], mybir.AluOpType.add
            )
            nc.vector.tensor_copy(tmp_t[:, 0:1], sel_t[:, 0:1])
            for t0, t1 in TILE_RANGES:
                tile = pool.tile([P, TILE_W], mybir.dt.float32)
                tw = t1 - t0
                nc.sync.dma_start(out=tile[:, :tw], in_=x[:, t0:t1])
                for b, lo, hi in windows:
                    llo = max(lo, t0)
                    lhi = min(hi, t1)
                    if llo < lhi:
                        base = (C_LOCAL * b) // 32 * 32
                        slab = tile[base : base + 32, llo - t0 : lhi - t0]
                        nc.vector.tensor_scalar_mul(
                            slab, slab, sel_t[base : base + 32, b : b + 1]
                        )
                nc.scalar.dma_start(out=y[:, t0:t1], in_=tile[:, :tw])
    return nc


def _split_multiwait(nc: bass.Bass) -> None:
    """This walrus codegen allows at most ONE sync-wait command per
    instruction.  Tile sometimes attaches several (e.g. a store waiting on
    both the fixup compute and the original load).  Hoist all but one wait
    onto standalone EventSemaphore instructions inserted just before the
    instruction on the same engine (engines execute their stream in order,
    so this preserves semantics).  We keep the compute-engine wait on DMA
    instructions (it completes last there) and hoist the DMA-queue waits.
    """
    ctr = [0]

    def mk_wait(engine, w):
        ctr[0] += 1
        ev = mybir.InstEventSemaphore(name=f"WSPLIT-{ctr[0]}")
        ev.engine = engine
        ev.sync_info = mybir.SyncInfo(on_wait=[w], on_update=[])
        return ev

    for f in nc.m.functions:
        for bb in f.blocks:
            new_insts = []
            changed = False
            for inst in bb.instructions:
                si = inst.sync_info
                ow = list(si.on_wait) if si is not None else []
                if len(ow) > 1:
                    dma_waits = [w for w in ow if "DMA" in (w.ant_name or "")]
                    other = [w for w in ow if w not in dma_waits]
                    keep = (other or dma_waits)[-1]
                    hoist = [w for w in ow if w is not keep]
                    for w in hoist:
                        new_insts.append(mk_wait(inst.engine, w))
                    inst.sync_info = mybir.SyncInfo(
                        on_wait=[keep], on_update=list(si.on_update)
                    )
                    changed = True
                new_insts.append(inst)
            if changed:
                bb.instructions = new_insts


def _get_program(starts: np.ndarray, widths: np.ndarray) -> bass.Bass:
    key = starts.tobytes() + widths.tobytes()
    prog = _program_cache.get(key)
    if prog is None:
        w = np.clip(widths, 1, MAX_MASK_WIDTH)
        # Per-b union of mask intervals (merge overlapping/adjacent)
        windows = []
        for b in range(B):
            ivs = sorted(
                (int(starts[m, b]), min(int(starts[m, b]) + int(w[m, b]), T))
                for m in range(starts.shape[0])
            )
            merged = [ivs[0]]
            for s, e in ivs[1:]:
                if s <= merged[-1][1]:
                    merged[-1] = (merged[-1][0], max(merged[-1][1], e))
                else:
                    merged.append((s, e))
            windows.extend((b, s, e) for s, e in merged if s < e)
        prog = _build_program(windows)
        _split_multiwait(prog)
        _program_cache[key] = prog
    return prog


def _run(x, starts, widths, trace=False, tmpdir=None):
    x = np.ascontiguousarray(x, dtype=np.float32)
    starts = np.asarray(starts, dtype=np.int32)
    widths = np.asarray(widths, dtype=np.int32)
    assert x.shape == (B, C, T), x.shape

    nc = _get_program(starts, widths)
    in_maps = [
        {
            "x": np.ascontiguousarray(
                x[:, k * C_LOCAL : (k + 1) * C_LOCAL, :]
            ).reshape(P, T)
        }
        for k in range(N_CORES)
    ]
    res = run_bass_kernel_spmd(
        nc, in_maps, list(range(N_CORES)), trace=trace, tmpdir=tmpdir
    )

    out = np.empty_like(x)
    for k in range(N_CORES):
        out[:, k * C_LOCAL : (k + 1) * C_LOCAL, :] = res.results[k]["y"].reshape(
            B, C_LOCAL, T
        )
    return out, res


def kernel(x, starts, widths):
    out, _ = _run(x, starts, widths, trace=False)
    return out



# revision 2
# speedup vs baseline: 1.3755x; 1.3755x over previous
"""Trainium2 Bass kernel for GPUTimeMask: zero out per-batch time windows.

Semantics (matches reference):
    out = x.copy();  for m, b:  out[b, :, s[m,b] : s[m,b]+clip(w[m,b],1,150)] = 0

Strategy (v2 — DRAM->DRAM streaming):
  - Shard x along the CHANNEL axis: 16 channels -> 2 per core across 8 cores.
    Every core holds ALL 64 batch rows, so the (runtime-valued) mask windows
    live at identical local coordinates on every core -> one SPMD program
    with window offsets specialized in at build time.
  - Per core the output is a byte-for-byte copy of the input except ~128
    tiny windows (<= 2 rows x 150 cols).  Instead of staging through SBUF,
    issue big DRAM->DRAM DMA copies: the plane [128, 60000] f32 is split
    into row groups, each a contiguous multi-MB chunk, alternated across
    the two HWDGE queues (qSP / qAct) with NO waits on the issue path.
    Every SDMA descriptor then does read+write inline, so HBM traffic is
    perfectly duplex-mixed from the first byte and runs at the ~435 GB/s
    SDMA ceiling with no SBUF pipeline, no buffer-reuse WARs, and no
    compute engines in the path.
  - The mask windows are then overwritten with zeros by tiny SWDGE DMAs
    (gpsimd queue) sourced from a memset SBUF tile.  Each fill waits (on
    the gpsimd sequencer only) for the semaphore of the row-group copy
    that covers its rows, so fills chase the copy stream and only the
    last group's handful of fills land after the final copy.
  - Raw bass (no TileContext): semaphores are placed by hand, each
    instruction carries at most one wait, and the only end-of-kernel cost
    is waiting for the terminal semaphore values.
  - Programs are cached keyed on (starts, widths) bytes.
"""

import sys

import numpy as np

for _p in ("/opt/trn_rl_repo",):
    if _p not in sys.path:
        sys.path.insert(0, _p)

import concourse.bass as bass
import concourse.mybir as mybir
from concourse.bass_utils import run_bass_kernel_spmd

B, C, T = 64, 16, 60000
MAX_MASK_WIDTH = 150
N_CORES = 8
C_LOCAL = C // N_CORES          # 2 channels per core
P = B * C_LOCAL                 # 128 rows: row = b * C_LOCAL + c_local

# Row groups for the big D2D copies.  8 rows = 4 batches per group: each
# group is a contiguous 8*60000*4 = 1.92 MB chunk -> 32 descriptors of
# 60 KB, spread over all 16 SDMA engines.  Even groups go on the qSP
# HWDGE queue, odd groups on qAct, so the two queues drain in lockstep
# and group completions arrive in ~index order for the fill chaser.
GROUP_ROWS = 8
N_GROUPS = P // GROUP_ROWS      # 16

_program_cache: dict[bytes, bass.Bass] = {}


def _merged_windows(starts: np.ndarray, widths: np.ndarray) -> list[list[tuple[int, int]]]:
    """Per-batch union of mask intervals (merge overlapping/adjacent)."""
    w = np.clip(widths, 1, MAX_MASK_WIDTH)
    out: list[list[tuple[int, int]]] = []
    for b in range(B):
        ivs = sorted(
            (int(starts[m, b]), min(int(starts[m, b]) + int(w[m, b]), T))
            for m in range(starts.shape[0])
        )
        merged = [ivs[0]]
        for s, e in ivs[1:]:
            if s <= merged[-1][1]:
                merged[-1] = (merged[-1][0], max(merged[-1][1], e))
            else:
                merged.append((s, e))
        out.append([(s, e) for s, e in merged if s < e])
    return out


def _build_program(windows: list[list[tuple[int, int]]]) -> bass.Bass:
    """windows[b]: merged (lo, hi) column ranges to zero; identical per core."""
    nc = bass.Bass()
    x = nc.declare_dram_parameter("x", [P, T], mybir.dt.float32, isOutput=False)
    y = nc.declare_dram_parameter("y", [P, T], mybir.dt.float32, isOutput=True)

    n_fills = sum(len(ws) for ws in windows)

    copy_sems = [nc.alloc_semaphore(f"copy_g{g}") for g in range(N_GROUPS)]
    fill_sem = nc.alloc_semaphore("fills")

    with nc.sbuf_tensor("zeros", [32, MAX_MASK_WIDTH + 2], mybir.dt.float32) as zsb:
        # Zero source for the window fills.  gpsimd issues the fills, so
        # same-engine program order makes the memset safe without a sem.
        nc.gpsimd.memset(zsb[:], 0.0)

        # Big D2D copies: no waits anywhere on the issue path.
        for g in range(N_GROUPS):
            r0, r1 = g * GROUP_ROWS, (g + 1) * GROUP_ROWS
            eng = nc.sync if g % 2 == 0 else nc.scalar
            eng.dma_start(out=y[r0:r1, :], in_=x[r0:r1, :]).then_inc(
                copy_sems[g], 16
            )

        # Window fills chase the copies on the SWDGE queue.
        for g in range(N_GROUPS):
            r0 = g * GROUP_ROWS
            nc.gpsimd.wait_ge(copy_sems[g], 16)
            for b in range(r0 // C_LOCAL, (r0 + GROUP_ROWS) // C_LOCAL):
                row = b * C_LOCAL
                for lo, hi in windows[b]:
                    nc.gpsimd.dma_start(
                        out=y[row : row + C_LOCAL, lo:hi],
                        in_=zsb[0:C_LOCAL, 0 : hi - lo],
                    ).then_inc(fill_sem, 16)

        # Terminal waits: the kernel may not retire until every DMA has
        # landed.  Queue FIFO order makes the last copy's sem imply the
        # earlier ones per queue, but the extra satisfied waits are ~free.
        for g in range(N_GROUPS):
            (nc.sync if g % 2 == 0 else nc.scalar).wait_ge(copy_sems[g], 16)
        nc.gpsimd.wait_ge(fill_sem, 16 * n_fills)

    return nc


def _get_program(starts: np.ndarray, widths: np.ndarray) -> bass.Bass:
    key = starts.tobytes() + widths.tobytes()
    prog = _program_cache.get(key)
    if prog is None:
        prog = _build_program(_merged_windows(starts, widths))
        _program_cache[key] = prog
    return prog


def _run(x, starts, widths, trace=False, tmpdir=None):
    x = np.ascontiguousarray(x, dtype=np.float32)
    starts = np.asarray(starts, dtype=np.int32)
    widths = np.asarray(widths, dtype=np.int32)
    assert x.shape == (B, C, T), x.shape

    nc = _get_program(starts, widths)
    in_maps = [
        {
            "x": np.ascontiguousarray(
                x[:, k * C_LOCAL : (k + 1) * C_LOCAL, :]
            ).reshape(P, T)
        }
        for k in range(N_CORES)
    ]
    res = run_bass_kernel_spmd(
        nc, in_maps, list(range(N_CORES)), trace=trace, tmpdir=tmpdir
    )

    out = np.empty_like(x)
    for k in range(N_CORES):
        out[:, k * C_LOCAL : (k + 1) * C_LOCAL, :] = res.results[k]["y"].reshape(
            B, C_LOCAL, T
        )
    return out, res


def kernel(x, starts, widths):
    out, _ = _run(x, starts, widths, trace=False)
    return out


# revision 3
# speedup vs baseline: 1.5454x; 1.1235x over previous
"""Trainium2 Bass kernel for GPUTimeMask: zero out per-batch time windows.

Semantics (matches reference):
    out = x.copy();  for m, b:  out[b, :, s[m,b] : s[m,b]+clip(w[m,b],1,150)] = 0

Strategy (v3 — DRAM->DRAM streaming on three queues):
  - Shard x along the CHANNEL axis: 16 channels -> 2 per core across 8 cores.
    Every core holds ALL 64 batch rows, so the (runtime-valued) mask windows
    live at identical local coordinates on every core -> one SPMD program
    with window offsets specialized in at build time.
  - Per core the output is a byte-for-byte copy of the input except ~128
    tiny windows (<= 2 rows x 150 cols).  Instead of staging through SBUF
    (which pins the kernel to the ~435 GB/s SBUF-AXI fabric ceiling), issue
    big DRAM->DRAM DMA copies: each SDMA descriptor reads and writes HBM
    inline, so HBM runs duplex (~640+ GB/s measured) with no SBUF pipeline,
    no buffer-reuse WARs, and no compute engines in the path.
  - The plane [128, 60000] f32 is split into 32 contiguous 4-row groups
    (960 KB each -> 16 descriptors of 60 KB, one per SDMA engine),
    round-robined over THREE issue queues: qSP + qAct (HWDGE) and the
    gpsimd SWDGE queue.  Three queues keep more descriptors in flight,
    lifting SDMA-engine occupancy versus two.
  - Mask windows are overwritten with zeros by tiny DMAs sourced from a
    memset SBUF tile.  Each queue fills the windows of its OWN groups,
    interleaved behind its copy stream with a 3-group lag: by the time the
    sequencer reaches "wait for group g's copy, then fill g's windows",
    that copy has long completed, so the waits almost never stall and no
    queue builds a fill backlog.  Only the last group's ~4 fills land
    after the final copy.
  - Raw bass (no TileContext): semaphores placed by hand, one wait per
    instruction, and the only end-of-kernel cost is the terminal waits.
  - Programs are cached keyed on (starts, widths) bytes.
"""

import sys

import numpy as np

for _p in ("/opt/trn_rl_repo",):
    if _p not in sys.path:
        sys.path.insert(0, _p)

import concourse.bass as bass
import concourse.mybir as mybir
from concourse.bass_utils import run_bass_kernel_spmd

B, C, T = 64, 16, 60000
MAX_MASK_WIDTH = 150
N_CORES = 8
C_LOCAL = C // N_CORES          # 2 channels per core
P = B * C_LOCAL                 # 128 rows: row = b * C_LOCAL + c_local

GROUP_ROWS = 4                  # 2 batches; contiguous 960 KB per group
N_GROUPS = P // GROUP_ROWS      # 32
N_QUEUES = 3                    # qSP, qAct (HWDGE) + gpsimd (SWDGE)
PRIME = 3                       # copies enqueued ahead of the first fill wait

_program_cache: dict[bytes, bass.Bass] = {}


def _merged_windows(starts: np.ndarray, widths: np.ndarray) -> list[list[tuple[int, int]]]:
    """Per-batch union of mask intervals (merge overlapping/adjacent)."""
    w = np.clip(widths, 1, MAX_MASK_WIDTH)
    out: list[list[tuple[int, int]]] = []
    for b in range(B):
        ivs = sorted(
            (int(starts[m, b]), min(int(starts[m, b]) + int(w[m, b]), T))
            for m in range(starts.shape[0])
        )
        merged = [ivs[0]]
        for s, e in ivs[1:]:
            if s <= merged[-1][1]:
                merged[-1] = (merged[-1][0], max(merged[-1][1], e))
            else:
                merged.append((s, e))
        out.append([(s, e) for s, e in merged if s < e])
    return out


def _build_program(windows: list[list[tuple[int, int]]]) -> bass.Bass:
    """windows[b]: merged (lo, hi) column ranges to zero; identical per core."""
    nc = bass.Bass()
    x = nc.declare_dram_parameter("x", [P, T], mybir.dt.float32, isOutput=False)
    y = nc.declare_dram_parameter("y", [P, T], mybir.dt.float32, isOutput=True)

    copy_sems = [nc.alloc_semaphore(f"copy_g{g}") for g in range(N_GROUPS)]
    fill_sems = [nc.alloc_semaphore(f"fills_q{q}") for q in range(N_QUEUES)]
    engines = [nc.sync, nc.scalar, nc.gpsimd]

    def group_fills(g):
        """(out_ap_args, n) column windows of group g as (row, lo, hi)."""
        out = []
        b0 = g * GROUP_ROWS // C_LOCAL
        for b in range(b0, b0 + GROUP_ROWS // C_LOCAL):
            for lo, hi in windows[b]:
                out.append((b * C_LOCAL, lo, hi))
        return out

    with nc.sbuf_tensor("zeros", [32, MAX_MASK_WIDTH + 2], mybir.dt.float32) as zsb:
        # Zero source for the window fills.  Every queue's fills read it;
        # gpsimd memsets it and the two HWDGE queues only reach their first
        # fill after a copy-sem wait that the memset long precedes, but add
        # an explicit handshake to be safe: gpsimd bumps each fill sem once.
        nc.gpsimd.memset(zsb[:], 0.0)
        for q in range(N_QUEUES):
            nc.gpsimd.sem_inc(fill_sems[q], 1)
        nc.sync.wait_ge(fill_sems[0], 1)
        nc.scalar.wait_ge(fill_sems[1], 1)

        qgroups = [[g for g in range(N_GROUPS) if g % N_QUEUES == q] for q in range(N_QUEUES)]
        n_fills_q = [0] * N_QUEUES

        for q, eng in enumerate(engines):
            gs = qgroups[q]

            def copy(g):
                r0, r1 = g * GROUP_ROWS, (g + 1) * GROUP_ROWS
                eng.dma_start(out=y[r0:r1, :], in_=x[r0:r1, :]).then_inc(
                    copy_sems[g], 16
                )

            def fill(g):
                eng.wait_ge(copy_sems[g], 16)
                for row, lo, hi in group_fills(g):
                    eng.dma_start(
                        out=y[row : row + C_LOCAL, lo:hi],
                        in_=zsb[0:C_LOCAL, 0 : hi - lo],
                    ).then_inc(fill_sems[q], 16)
                    n_fills_q[q] += 1

            for g in gs[:PRIME]:
                copy(g)
            for i, g in enumerate(gs):
                if i + PRIME < len(gs):
                    copy(gs[i + PRIME])
                fill(g)

        # Terminal waits: the kernel may not retire until every DMA landed.
        for q, eng in enumerate(engines):
            for g in qgroups[q]:
                eng.wait_ge(copy_sems[g], 16)
            eng.wait_ge(fill_sems[q], 16 * n_fills_q[q] + 1)

    return nc


def _get_program(starts: np.ndarray, widths: np.ndarray) -> bass.Bass:
    key = starts.tobytes() + widths.tobytes()
    prog = _program_cache.get(key)
    if prog is None:
        prog = _build_program(_merged_windows(starts, widths))
        _program_cache[key] = prog
    return prog


def _run(x, starts, widths, trace=False, tmpdir=None):
    x = np.ascontiguousarray(x, dtype=np.float32)
    starts = np.asarray(starts, dtype=np.int32)
    widths = np.asarray(widths, dtype=np.int32)
    assert x.shape == (B, C, T), x.shape

    nc = _get_program(starts, widths)
    in_maps = [
        {
            "x": np.ascontiguousarray(
                x[:, k * C_LOCAL : (k + 1) * C_LOCAL, :]
            ).reshape(P, T)
        }
        for k in range(N_CORES)
    ]
    res = run_bass_kernel_spmd(
        nc, in_maps, list(range(N_CORES)), trace=trace, tmpdir=tmpdir
    )

    out = np.empty_like(x)
    for k in range(N_CORES):
        out[:, k * C_LOCAL : (k + 1) * C_LOCAL, :] = res.results[k]["y"].reshape(
            B, C_LOCAL, T
        )
    return out, res


def kernel(x, starts, widths):
    out, _ = _run(x, starts, widths, trace=False)
    return out


# revision 4
# speedup vs baseline: 1.5655x; 1.0130x over previous
"""Trainium2 Bass kernel for GPUTimeMask: zero out per-batch time windows.

Semantics (matches reference):
    out = x.copy();  for m, b:  out[b, :, s[m,b] : s[m,b]+clip(w[m,b],1,150)] = 0

Strategy (v3 — DRAM->DRAM streaming on three queues):
  - Shard x along the CHANNEL axis: 16 channels -> 2 per core across 8 cores.
    Every core holds ALL 64 batch rows, so the (runtime-valued) mask windows
    live at identical local coordinates on every core -> one SPMD program
    with window offsets specialized in at build time.
  - Per core the output is a byte-for-byte copy of the input except ~128
    tiny windows (<= 2 rows x 150 cols).  Instead of staging through SBUF
    (which pins the kernel to the ~435 GB/s SBUF-AXI fabric ceiling), issue
    big DRAM->DRAM DMA copies: each SDMA descriptor reads and writes HBM
    inline, so HBM runs duplex (~640+ GB/s measured) with no SBUF pipeline,
    no buffer-reuse WARs, and no compute engines in the path.
  - The plane [128, 60000] f32 is split into 32 contiguous 4-row groups
    (960 KB each -> 16 descriptors of 60 KB, one per SDMA engine),
    round-robined over THREE issue queues: qSP + qAct (HWDGE) and the
    gpsimd SWDGE queue.  Three queues keep more descriptors in flight,
    lifting SDMA-engine occupancy versus two.
  - Mask windows are overwritten with zeros by tiny DMAs sourced from a
    memset SBUF tile.  Each queue fills the windows of its OWN groups,
    interleaved behind its copy stream with a 3-group lag: by the time the
    sequencer reaches "wait for group g's copy, then fill g's windows",
    that copy has long completed, so the waits almost never stall and no
    queue builds a fill backlog.  Only the last group's ~4 fills land
    after the final copy.
  - Raw bass (no TileContext): semaphores placed by hand, one wait per
    instruction, and the only end-of-kernel cost is the terminal waits.
  - Programs are cached keyed on (starts, widths) bytes.
"""

import sys

import numpy as np

for _p in ("/opt/trn_rl_repo",):
    if _p not in sys.path:
        sys.path.insert(0, _p)

import concourse.bass as bass
import concourse.mybir as mybir
from concourse.bass_utils import run_bass_kernel_spmd

B, C, T = 64, 16, 60000
MAX_MASK_WIDTH = 150
N_CORES = 8
C_LOCAL = C // N_CORES          # 2 channels per core
P = B * C_LOCAL                 # 128 rows: row = b * C_LOCAL + c_local

GROUP_ROWS = 4                  # 2 batches; contiguous 960 KB per group
N_GROUPS = P // GROUP_ROWS      # 32
N_QUEUES = 3                    # qSP, qAct (HWDGE) + gpsimd (SWDGE)
PRIME = 3                       # copies enqueued ahead of the first fill wait

_program_cache: dict[bytes, bass.Bass] = {}


def _merged_windows(starts: np.ndarray, widths: np.ndarray) -> list[list[tuple[int, int]]]:
    """Per-batch union of mask intervals (merge overlapping/adjacent)."""
    w = np.clip(widths, 1, MAX_MASK_WIDTH)
    out: list[list[tuple[int, int]]] = []
    for b in range(B):
        ivs = sorted(
            (int(starts[m, b]), min(int(starts[m, b]) + int(w[m, b]), T))
            for m in range(starts.shape[0])
        )
        merged = [ivs[0]]
        for s, e in ivs[1:]:
            if s <= merged[-1][1]:
                merged[-1] = (merged[-1][0], max(merged[-1][1], e))
            else:
                merged.append((s, e))
        out.append([(s, e) for s, e in merged if s < e])
    return out


def _build_program(windows: list[list[tuple[int, int]]]) -> bass.Bass:
    """windows[b]: merged (lo, hi) column ranges to zero; identical per core."""
    nc = bass.Bass()
    x = nc.declare_dram_parameter("x", [P, T], mybir.dt.float32, isOutput=False)
    y = nc.declare_dram_parameter("y", [P, T], mybir.dt.float32, isOutput=True)

    copy_sems = [nc.alloc_semaphore(f"copy_g{g}") for g in range(N_GROUPS)]
    fill_sems = [nc.alloc_semaphore(f"fills_q{q}") for q in range(N_QUEUES)]
    engines = [nc.sync, nc.scalar, nc.gpsimd]

    def group_fills(g):
        """(out_ap_args, n) column windows of group g as (row, lo, hi)."""
        out = []
        b0 = g * GROUP_ROWS // C_LOCAL
        for b in range(b0, b0 + GROUP_ROWS // C_LOCAL):
            for lo, hi in windows[b]:
                out.append((b * C_LOCAL, lo, hi))
        return out

    with nc.sbuf_tensor("zeros", [32, MAX_MASK_WIDTH + 2], mybir.dt.float32) as zsb:
        # Zero source for the window fills.  Every queue's fills read it;
        # gpsimd memsets it and the two HWDGE queues only reach their first
        # fill after a copy-sem wait that the memset long precedes, but add
        # an explicit handshake to be safe: gpsimd bumps each fill sem once.
        nc.gpsimd.memset(zsb[:], 0.0)
        for q in range(N_QUEUES):
            nc.gpsimd.sem_inc(fill_sems[q], 1)
        nc.sync.wait_ge(fill_sems[0], 1)
        nc.scalar.wait_ge(fill_sems[1], 1)

        qgroups = [[g for g in range(N_GROUPS) if g % N_QUEUES == q] for q in range(N_QUEUES)]
        n_fills_q = [0] * N_QUEUES

        for q, eng in enumerate(engines):
            gs = qgroups[q]

            def copy(g):
                r0, r1 = g * GROUP_ROWS, (g + 1) * GROUP_ROWS
                # 30 KB descriptors (32 per group, 2 waves over the 16 SDMA
                # engines) pipeline the per-descriptor HBM read->write turn
                # better than the default 60 KB split.
                eng.dma_start(
                    out=y[r0:r1, :], in_=x[r0:r1, :], max_dma_last_dim=30720
                ).then_inc(copy_sems[g], 16)

            def fill(g):
                eng.wait_ge(copy_sems[g], 16)
                for row, lo, hi in group_fills(g):
                    eng.dma_start(
                        out=y[row : row + C_LOCAL, lo:hi],
                        in_=zsb[0:C_LOCAL, 0 : hi - lo],
                    ).then_inc(fill_sems[q], 16)
                    n_fills_q[q] += 1

            for g in gs[:PRIME]:
                copy(g)
            for i, g in enumerate(gs):
                if i + PRIME < len(gs):
                    copy(gs[i + PRIME])
                fill(g)

        # Terminal waits: the kernel may not retire until every DMA landed.
        for q, eng in enumerate(engines):
            for g in qgroups[q]:
                eng.wait_ge(copy_sems[g], 16)
            eng.wait_ge(fill_sems[q], 16 * n_fills_q[q] + 1)

    return nc


def _get_program(starts: np.ndarray, widths: np.ndarray) -> bass.Bass:
    key = starts.tobytes() + widths.tobytes()
    prog = _program_cache.get(key)
    if prog is None:
        prog = _build_program(_merged_windows(starts, widths))
        _program_cache[key] = prog
    return prog


def _run(x, starts, widths, trace=False, tmpdir=None):
    x = np.ascontiguousarray(x, dtype=np.float32)
    starts = np.asarray(starts, dtype=np.int32)
    widths = np.asarray(widths, dtype=np.int32)
    assert x.shape == (B, C, T), x.shape

    nc = _get_program(starts, widths)
    in_maps = [
        {
            "x": np.ascontiguousarray(
                x[:, k * C_LOCAL : (k + 1) * C_LOCAL, :]
            ).reshape(P, T)
        }
        for k in range(N_CORES)
    ]
    res = run_bass_kernel_spmd(
        nc, in_maps, list(range(N_CORES)), trace=trace, tmpdir=tmpdir
    )

    out = np.empty_like(x)
    for k in range(N_CORES):
        out[:, k * C_LOCAL : (k + 1) * C_LOCAL, :] = res.results[k]["y"].reshape(
            B, C_LOCAL, T
        )
    return out, res


def kernel(x, starts, widths):
    out, _ = _run(x, starts, widths, trace=False)
    return out
